# revision 18
# baseline (speedup 1.0000x reference)
"""Trainium2 Bass kernel for padded/ragged multi-head attention.

Problem shape (hardcoded, matches the grading harness):
  B=8 sequences, S=1024 padded length, VALID=512 valid tokens/seq,
  H=1024 hidden, NH=16 heads, HD=64 head dim, T=B*VALID=4096 tokens.

Sharding: pure data parallel, one batch per NeuronCore (8 cores).

Because the valid tokens of each sequence sit at positions [0, VALID) and all
padded key positions carry a -1e9 additive bias (exp underflows to exactly 0
in fp32), the padded-attention computation reduces exactly to dense attention
over each sequence's 512 valid tokens.  Padding is never materialized.

Per-core pipeline (feature-major, zero transposes):
  1. Q^T/K^T = W_qk^T X^T (features on partitions), interleaved q,k so head
     pairs complete early.
  2. RoPE: roped = q*cos + R(q)*sin with R a +-1 signed-permutation matmul.
  3. scores^T per head emitted inside phase 1 as soon as the head's q,k tiles
     are roped — the scalar-engine exp (the 2nd-most-expensive engine total)
     overlaps the projection matmuls instead of gating the ctx matmuls.
     Keys-on-partitions layout fuses exp's 1/sqrt(HD) scale and per-key bias
     into the single activation op.
  4. V in token-major layout with an appended ones column; the ctx matmul
     yields ctx^T and the softmax denominator in one PSUM tile.
  5. Denominators collected into a (16,512) tile; batched
     reciprocal_approx_fast; per-head partition-broadcast via a stride-0
     free-dim SBUF->SBUF DMA; one DVE multiply normalizes each head.
  6. ctx^T feeds o_proj as lhsT directly; the Tile scheduler interleaves
     o_proj accumulation chunks into phase 5 as ctx tiles complete.

Matmul operands are fp16 (1 cycle/row on the PE, fast weight load, fp32 PSUM
accumulation; end-to-end relative error ~6e-4).
"""

import sys
import numpy as np

sys.path.insert(0, "/opt/trn_rl_repo")

B = 8
S = 1024
H = 1024
NH = 16
HD = 64
VALID = 512
P = 128
KC = H // P            # 8 contraction chunks of 128
QK_TILES = 2 * H // P  # 16 feature-major tiles for Q^T and K^T
TC = VALID // P        # 4 token chunks
VW = HD + 1            # V columns per head incl. ones column

_CACHE = {}


def _build(with_qkv_bias):
    import concourse.mybir as mybir
    import concourse.tile as tile
    from concourse import bacc
    from contextlib import ExitStack

    F32 = mybir.dt.float32
    F16 = mybir.dt.float16
    EXP = mybir.ActivationFunctionType.Exp

    nc = bacc.Bacc()
    xT = nc.declare_dram_parameter("xT", [P, KC, VALID], F16, isOutput=False)
    wqk = nc.declare_dram_parameter("wqk", [QK_TILES, P, KC, P], F16, isOutput=False)
    wv = nc.declare_dram_parameter("wv", [2, P, KC, 512], F16, isOutput=False)
    wo = nc.declare_dram_parameter("wo", [2, P, KC, 512], F16, isOutput=False)
    cos2 = nc.declare_dram_parameter("cos2", [P, VALID], F16, isOutput=False)
    sin2 = nc.declare_dram_parameter("sin2", [P, VALID], F16, isOutput=False)
    rot = nc.declare_dram_parameter("rot", [P, P], F16, isOutput=False)
    biask = nc.declare_dram_parameter("biask", [P, TC], F32, isOutput=False)
    ones16 = nc.declare_dram_parameter("ones16", [P, NH], F16, isOutput=False)
    if with_qkv_bias:
        qb_rope = nc.declare_dram_parameter("qb_rope", [P, QK_TILES, VALID], F32, isOutput=False)
        vbias = nc.declare_dram_parameter("vbias", [P, KC], F32, isOutput=False)
    out = nc.declare_dram_parameter("out", [VALID, H], F32, isOutput=True)

    with tile.TileContext(nc) as tc:
        with ExitStack() as ctx:
            consts = ctx.enter_context(tc.tile_pool(name="consts", bufs=1))
            xpool = ctx.enter_context(tc.tile_pool(name="x", bufs=1))
            # per-m weight tiles, loaded in consumption order, deep prefetch
            wqk_pool = ctx.enter_context(tc.tile_pool(name="wqk", bufs=6))
            wno_pool = ctx.enter_context(tc.tile_pool(name="wno", bufs=2))
            qsb_pool = ctx.enter_context(tc.tile_pool(name="qsb", bufs=3))
            qk_pool = ctx.enter_context(tc.tile_pool(name="qk", bufs=NH // 2 + NH))
            v_pool = ctx.enter_context(tc.tile_pool(name="v", bufs=TC))
            e_pool = ctx.enter_context(tc.tile_pool(name="e", bufs=NH * TC))
            ctx_pool = ctx.enter_context(tc.tile_pool(name="ctx", bufs=KC))
            tmp_pool = ctx.enter_context(tc.tile_pool(name="tmp", bufs=3))
            den_pool = ctx.enter_context(tc.tile_pool(name="den", bufs=4))
            rb_pool = ctx.enter_context(tc.tile_pool(name="rb", bufs=3))
            csb_pool = ctx.enter_context(tc.tile_pool(name="csb", bufs=4))
            o_pool = ctx.enter_context(tc.tile_pool(name="o", bufs=3))
            proj_ps = ctx.enter_context(tc.tile_pool(name="pps", bufs=2, space="PSUM"))
            rot_ps = ctx.enter_context(tc.tile_pool(name="rps", bufs=2, space="PSUM"))
            s_ps = ctx.enter_context(tc.tile_pool(name="sps", bufs=2, space="PSUM"))
            c_ps = ctx.enter_context(tc.tile_pool(name="cps", bufs=2, space="PSUM"))

            # pin the activation table to natural_log_exp_and_others (set 6:
            # exp, ln, copy, identity all present) so the table-load pass
            # doesn't thrash between per-func sets
            nc.scalar.add_instruction(
                mybir.InstLoadActFuncSet(
                    name=nc.get_next_instruction_name(), ins=[], outs=[],
                    act_func_set_id=6,
                )
            )

            # Each HWDGE trigger costs ~0.65us on its sequencer, serialized
            # per ring.  Split x across both rings and issue the first
            # matmul's dependencies (x first half + first weight tile) as the
            # very first trigger on each ring so the PE starts ~10us earlier.
            xa1 = xpool.tile([P, 2, VALID], F16, tag="xa1", name="xa1")
            nc.scalar.dma_start(xa1[:], xT[:, 0:2, :])
            xa2 = xpool.tile([P, 2, VALID], F16, tag="xa2", name="xa2")
            nc.scalar.dma_start(xa2[:], xT[:, 2:4, :])

            # warmup: dummy matmuls on a zeroed scratch tile keep the PE busy
            # while the first input DMAs land, so HAM is at full clock when
            # real work starts
            wu = xpool.tile([P, VALID], F16, tag="wu", name="wu")
            nc.gpsimd.memset(wu[:], 0.0)
            wups = proj_ps.tile([P, VALID], F32, tag="pps", name="wups")
            for _ in range(8):
                nc.tensor.matmul(wups[:], wu[:, :P], wu[:], start=True, stop=True)

            # small consts on the scalar HWDGE ring; bulk loads on sync ring
            cos_t = consts.tile([P, VALID], F16, tag="cos")
            sin_t = consts.tile([P, VALID], F16, tag="sin")
            rot_t = consts.tile([P, P], F16, tag="rot")
            bias_t = consts.tile([P, TC], F32, tag="biask")
            ones_t = consts.tile([P, NH], F16, tag="ones16")
            nc.scalar.dma_start(cos_t[:], cos2[:])
            nc.scalar.dma_start(sin_t[:], sin2[:])
            nc.scalar.dma_start(rot_t[:], rot[:])
            nc.scalar.dma_start(bias_t[:], biask[:])
            nc.scalar.dma_start(ones_t[:], ones16[:])
            if with_qkv_bias:
                qb_t = consts.tile([P, QK_TILES, VALID], F32, tag="qb")
                nc.scalar.dma_start(qb_t[:], qb_rope[:])
                vb_t = consts.tile([P, KC], F32, tag="vb")
                nc.scalar.dma_start(vb_t[:], vbias[:])

            # first weight tile, then x second half, on the sync ring
            wm_tiles = {}
            wm0 = wqk_pool.tile([P, KC, P], F16, tag="wqk", name="wm0")
            nc.sync.dma_start(wm0[:], wqk[0])
            wm_tiles[0] = wm0
            xb = xpool.tile([P, 4, VALID], F16, tag="xb", name="xb")
            nc.sync.dma_start(xb[:], xT[:, 4:8, :])

            def x_sl(kc):
                if kc < 2:
                    return xa1[:, kc, :]
                if kc < 4:
                    return xa2[:, kc - 2, :]
                return xb[:, kc - 4, :]

            escale = 1.0 / np.sqrt(HD)
            qk_tiles = {}
            kpads = {}
            e_tiles = {}
            # zero-padded K-tile halves: full-128-partition lhsT for the
            # scores matmuls enables fast weight load (the zero half
            # multiplies the other head's q rows by 0)
            for i in range(NH // 2):
                ka = qk_pool.tile([P, VALID], F16, tag="qk", name=f"kpa{i}")
                kb = qk_pool.tile([P, VALID], F16, tag="qk", name=f"kpb{i}")
                nc.gpsimd.memset(ka[HD:, :], 0.0)
                nc.gpsimd.memset(kb[:HD, :], 0.0)
                kpads[i] = (ka, kb)

            def emit_rot(pend):
                q_sb, m = pend
                rp = rot_ps.tile([P, VALID], F32, tag="rps", name=f"rp{m}")
                nc.tensor.matmul(rp[:], rot_t[:], q_sb[:], start=True, stop=True)
                t1 = tmp_pool.tile([P, VALID], F16, tag="t1", name=f"t1_{m}")
                nc.vector.tensor_mul(t1[:], q_sb[:], cos_t[:])
                t2 = tmp_pool.tile([P, VALID], F16, tag="t2", name=f"t2_{m}")
                nc.vector.tensor_mul(t2[:], rp[:], sin_t[:])
                if m >= NH // 2:
                    # K tile: write the roped halves into the zero-padded pair
                    ka, kb = kpads[m - NH // 2]
                    if with_qkv_bias:
                        t3 = tmp_pool.tile([P, VALID], F32, tag="t3", name=f"t3_{m}")
                        nc.vector.tensor_add(t3[:], t1[:], t2[:])
                        nc.vector.tensor_add(t3[:], t3[:], qb_t[:, m, :])
                        nc.vector.tensor_copy(ka[:HD, :], t3[:HD, :])
                        nc.vector.tensor_copy(kb[HD:, :], t3[HD:, :])
                    else:
                        nc.vector.tensor_add(ka[:HD, :], t1[:HD, :], t2[:HD, :])
                        nc.vector.tensor_add(kb[HD:, :], t1[HD:, :], t2[HD:, :])
                else:
                    qkm = qk_pool.tile([P, VALID], F16, tag="qk", name=f"qk{m}")
                    if with_qkv_bias:
                        t3 = tmp_pool.tile([P, VALID], F32, tag="t3", name=f"t3_{m}")
                        nc.vector.tensor_add(t3[:], t1[:], t2[:])
                        nc.vector.tensor_add(qkm[:], t3[:], qb_t[:, m, :])
                    else:
                        nc.vector.tensor_add(qkm[:], t1[:], t2[:])
                    qk_tiles[m] = qkm

            def emit_scores(pair):
                qt = qk_tiles[pair]
                for h in (2 * pair, 2 * pair + 1):
                    kp = kpads[pair][h % 2]
                    for j in range(TC):
                        sp = s_ps.tile([P, VALID], F32, tag="sps", name=f"s{h}_{j}")
                        nc.tensor.matmul(
                            sp[:],
                            kp[:, j * P : (j + 1) * P],
                            qt[:],
                            start=True, stop=True,
                        )
                        ej = e_pool.tile([P, VALID], F16, tag="e", name=f"e{h}_{j}")
                        nc.scalar.activation(
                            ej[:], sp[:], EXP, bias=bias_t[:, j : j + 1], scale=escale
                        )
                        e_tiles[(h, j)] = ej

            # ---- Phase B: QK projection + RoPE + scores/exp, interleaved ----
            order = [m for pair in range(NH // 2) for m in (pair, NH // 2 + pair)]
            pend = None
            for m in order:
                if m in wm_tiles:
                    wm = wm_tiles[m]
                else:
                    wm = wqk_pool.tile([P, KC, P], F16, tag="wqk", name=f"wm{m}")
                    nc.sync.dma_start(wm[:], wqk[m])
                ps = proj_ps.tile([P, VALID], F32, tag="pps", name=f"ps{m}")
                for kc in range(KC):
                    nc.tensor.matmul(
                        ps[:], wm[:, kc, :], x_sl(kc),
                        start=(kc == 0), stop=(kc == KC - 1),
                    )
                q_sb = qsb_pool.tile([P, VALID], F16, tag="qsb", name=f"qsb{m}")
                nc.vector.tensor_copy(q_sb[:], ps[:])
                if pend is not None:
                    pm = pend[1]
                    emit_rot(pend)
                    if pm >= NH // 2:
                        emit_scores(pm - NH // 2)
                pend = (q_sb, m)
            emit_rot(pend)
            emit_scores(NH // 2 - 1)

            # ---- Phase C: V projection into token-major augmented layout ----
            v_tiles = [v_pool.tile([P, NH, VW], F16, tag="v", name=f"v{t}") for t in range(TC)]
            for t in range(TC):
                nc.scalar.copy(v_tiles[t][:, :, HD], ones_t[:])
            wvns = []
            for n in range(2):
                wvn = wno_pool.tile([P, KC, 512], F16, tag="wno", name=f"wv{n}")
                nc.sync.dma_start(wvn[:], wv[n])
                wvns.append(wvn)
            for t in range(TC):
                for n in range(2):
                    ps = proj_ps.tile([P, 512], F32, tag="pps", name=f"vps{n}{t}")
                    for kc in range(KC):
                        nc.tensor.matmul(
                            ps[:], x_sl(kc)[:, t * P : (t + 1) * P], wvns[n][:, kc, :],
                            start=(kc == 0), stop=(kc == KC - 1),
                        )
                    nc.scalar.copy(
                        v_tiles[t][:, 8 * n : 8 * n + 8, :HD],
                        ps[:].rearrange("p (h c) -> p h c", c=HD),
                    )

            # ---- Phase D: ctx matmuls + batched-denominator normalize ----
            ctx_tiles = [ctx_pool.tile([P, VALID], F16, tag="ctx", name=f"ctx{m}") for m in range(KC)]
            cps = {}
            rrs = {}
            NB = 2  # heads per psum-pool alternation group

            def emit_norm(h):
                rb = rb_pool.tile([HD, VALID], F32, tag="rb", name=f"rb{h}")
                rr = rrs.pop(h)
                nc.gpsimd.partition_broadcast(rb[:], rr[:])
                csb = cps.pop(h)
                dst = ctx_tiles[h // 2][(h % 2) * HD : (h % 2) * HD + HD, :]
                if with_qkv_bias:
                    tmpc = tmp_pool.tile([HD, VALID], F32, tag="tc", name=f"tc{h}")
                    nc.vector.tensor_mul(tmpc[:], csb[:HD, :], rb[:])
                    nc.scalar.activation(
                        dst, tmpc[:], mybir.ActivationFunctionType.Identity,
                        bias=vb_t[(h % 2) * HD : (h % 2) * HD + HD, h // 2 : h // 2 + 1],
                    )
                else:
                    nc.vector.tensor_mul(dst, csb[:HD, :], rb[:])

            for h in range(NH):
                # alternate between two PSUM pools (rot pool is idle now) so
                # 4 ctx accumulators stay live across the normalize pipeline
                cpool, ctag = [(c_ps, "cps"), (rot_ps, "rps")][(h // NB) % 2]
                cp = cpool.tile([VW, VALID], F32, tag=ctag, name=f"c{h}")
                for j in range(TC):
                    nc.tensor.matmul(
                        cp[:], v_tiles[j][:, h, :], e_tiles[(h, j)][:],
                        start=(j == 0), stop=(j == TC - 1),
                    )
                # evacuate the accumulator through the vector engine so the
                # PSUM bank frees after one op; normalize chain reads SBUF
                csb = csb_pool.tile([VW, VALID], F32, tag="csb", name=f"csb{h}")
                nc.vector.tensor_copy(csb[:], cp[:])
                # reciprocal via scalar engine: 1/d = exp(-ln(d)); DVE's exact
                # reciprocal is ~6 cycles/element and would dominate the DVE
                lg = den_pool.tile([1, VALID], F32, tag="lg", name=f"lg{h}")
                nc.scalar.activation(lg[:], csb[HD : HD + 1, :],
                                     mybir.ActivationFunctionType.Ln)
                rr = den_pool.tile([1, VALID], F32, tag="rr", name=f"rr{h}")
                nc.scalar.activation(rr[:], lg[:],
                                     mybir.ActivationFunctionType.Exp, scale=-1.0)
                cps[h] = csb
                rrs[h] = rr
                if h % NB == NB - 1:
                    for hh in range(h - NB + 1, h + 1):
                        emit_norm(hh)

            # ---- Phase E: output projection ----
            for n in range(2):
                won = wno_pool.tile([P, KC, 512], F16, tag="wno", name=f"wo{n}")
                nc.sync.dma_start(won[:], wo[n])
                for t in range(TC):
                    pool_ = proj_ps if (t % 2 == 0) else s_ps
                    tag_ = "pps" if (t % 2 == 0) else "sps"
                    ps = pool_.tile([P, 512], F32, tag=tag_, name=f"ops{n}{t}")
                    for m in range(KC):
                        nc.tensor.matmul(
                            ps[:], ctx_tiles[m][:, t * P : (t + 1) * P], won[:, m, :],
                            start=(m == 0), stop=(m == KC - 1),
                        )
                    ot = o_pool.tile([P, 512], F32, tag="o", name=f"o{n}{t}")
                    nc.scalar.copy(ot[:], ps[:])
                    nc.sync.dma_start(
                        out[t * P : (t + 1) * P, n * 512 : (n + 1) * 512], ot[:]
                    )

    nc.compile()
    return nc


def _get_nc(with_qkv_bias):
    key = bool(with_qkv_bias)
    if key not in _CACHE:
        _CACHE[key] = _build(key)
    return _CACHE[key]


def _rot_matrix():
    # R such that (R.T @ q)[d] == rotate_half(q)[d], block-diagonal per head
    r64 = np.zeros((HD, HD), np.float32)
    half = HD // 2
    for d in range(half):
        r64[d + half, d] = -1.0  # dest d < 32 gets -q[d+32]
        r64[d, d + half] = 1.0   # dest d >= 32 gets  q[d-32]
    r = np.zeros((P, P), np.float32)
    r[:HD, :HD] = r64
    r[HD:, HD:] = r64
    return r


def _to_tiles_kxm(w, ncols):
    """(H, F) weight -> (F//ncols, P, KC, ncols) fp16, contiguous."""
    F = w.shape[1]
    t = w.reshape(KC, P, F // ncols, ncols).transpose(2, 1, 0, 3)
    return np.ascontiguousarray(t.astype(np.float16))


def kernel(hidden_states, cos, sin, attention_bias, qkv_w, qkv_b, o_w, o_b,
           indices, batch, seqlen, _trace=False):
    from concourse.bass_utils import run_bass_kernel_spmd

    hidden_states = np.asarray(hidden_states, dtype=np.float32)
    cos = np.asarray(cos, dtype=np.float32)
    sin = np.asarray(sin, dtype=np.float32)
    attention_bias = np.asarray(attention_bias, dtype=np.float32)
    qkv_w = np.asarray(qkv_w, dtype=np.float32)
    qkv_b = np.asarray(qkv_b, dtype=np.float32)
    o_w = np.asarray(o_w, dtype=np.float32)
    o_b = np.asarray(o_b, dtype=np.float32)
    indices = np.asarray(indices)
    batch = int(batch)
    seqlen = int(seqlen)

    with_bias = bool(np.any(qkv_b))

    pos = indices.astype(np.int64)
    b_of = pos // seqlen
    s_of = pos % seqlen

    wqk2 = _to_tiles_kxm(qkv_w[:, : 2 * H], P)        # (16, P, KC, P)
    wv2 = _to_tiles_kxm(qkv_w[:, 2 * H :], 512)       # (2, P, KC, 512)
    wo2 = _to_tiles_kxm(o_w, 512)                     # (2, P, KC, 512)
    rot = _rot_matrix().astype(np.float16)
    ones16 = np.ones((P, NH), np.float16)

    in_maps = []
    tok_idx = []
    for b in range(batch):
        idx = np.nonzero(b_of == b)[0]
        assert len(idx) == VALID, f"batch {b} has {len(idx)} valid tokens"
        tok_idx.append(idx)
        xT2 = np.ascontiguousarray(
            hidden_states[idx].T.reshape(KC, P, VALID).transpose(1, 0, 2)
            .astype(np.float16)
        )
        cosT = cos[idx, 0, :].T  # (HD, VALID)
        sinT = sin[idx, 0, :].T
        cos2 = np.ascontiguousarray(
            np.concatenate([cosT, cosT], axis=0).astype(np.float16))
        sin2 = np.ascontiguousarray(
            np.concatenate([sinT, sinT], axis=0).astype(np.float16))
        bias_b = attention_bias[b, 0, 0, s_of[idx]].astype(np.float32)  # (VALID,)
        biask = np.ascontiguousarray(bias_b.reshape(TC, P).T)  # (P, TC)
        m = {
            "xT": xT2, "wqk": wqk2, "wv": wv2, "wo": wo2,
            "cos2": cos2, "sin2": sin2, "rot": rot, "biask": biask,
            "ones16": ones16,
        }
        if with_bias:
            bq = qkv_b[: 2 * H]
            cos_full = np.tile(cosT, (2 * H // HD, 1))  # (2H, VALID)
            sin_full = np.tile(sinT, (2 * H // HD, 1))
            rot_bq = bq.reshape(-1, 2, HD // 2)[:, ::-1, :].copy()
            rot_bq[:, 0, :] *= -1.0
            rot_bq = rot_bq.reshape(-1)
            qb = (bq[:, None] * cos_full + rot_bq[:, None] * sin_full)
            qb = qb.reshape(QK_TILES, P, VALID).transpose(1, 0, 2)
            m["qb_rope"] = np.ascontiguousarray(qb.astype(np.float32))
            bv = qkv_b[2 * H :].astype(np.float32)
            m["vbias"] = np.ascontiguousarray(bv.reshape(KC, P).T)
        in_maps.append(m)

    nc = _get_nc(with_bias)
    res = run_bass_kernel_spmd(nc, in_maps, core_ids=list(range(B)), trace=_trace)

    T = hidden_states.shape[0]
    out_full = np.empty((T, H), np.float32)
    for b in range(batch):
        out_full[tok_idx[b]] = res.results[b]["out"]
    if np.any(o_b):
        out_full += o_b[None, :]
    if _trace:
        kernel.last_exec_time_ns = res.exec_time_ns
        kernel.last_results = res
    return out_full


# revision 19
# speedup vs baseline: 1.0165x; 1.0165x over previous
"""Trainium2 Bass kernel for padded/ragged multi-head attention.

Problem shape (hardcoded, matches the grading harness):
  B=8 sequences, S=1024 padded length, VALID=512 valid tokens/seq,
  H=1024 hidden, NH=16 heads, HD=64 head dim, T=B*VALID=4096 tokens.

Sharding: pure data parallel, one batch per NeuronCore (8 cores).

Because the valid tokens of each sequence sit at positions [0, VALID) and all
padded key positions carry a -1e9 additive bias (exp underflows to exactly 0
in fp32), the padded-attention computation reduces exactly to dense attention
over each sequence's 512 valid tokens.  Padding is never materialized.

Per-core pipeline (feature-major, zero transposes):
  1. Q^T/K^T = W_qk^T X^T (features on partitions), interleaved q,k so head
     pairs complete early.
  2. RoPE: roped = q*cos + R(q)*sin with R a +-1 signed-permutation matmul.
  3. scores^T per head emitted inside phase 1 as soon as the head's q,k tiles
     are roped — the scalar-engine exp (the 2nd-most-expensive engine total)
     overlaps the projection matmuls instead of gating the ctx matmuls.
     Keys-on-partitions layout fuses exp's 1/sqrt(HD) scale and per-key bias
     into the single activation op.
  4. V in token-major layout with an appended ones column; the ctx matmul
     yields ctx^T and the softmax denominator in one PSUM tile.
  5. Denominators collected into a (16,512) tile; batched
     reciprocal_approx_fast; per-head partition-broadcast via a stride-0
     free-dim SBUF->SBUF DMA; one DVE multiply normalizes each head.
  6. ctx^T feeds o_proj as lhsT directly; the Tile scheduler interleaves
     o_proj accumulation chunks into phase 5 as ctx tiles complete.

Matmul operands are fp16 (1 cycle/row on the PE, fast weight load, fp32 PSUM
accumulation; end-to-end relative error ~6e-4).
"""

import sys
import numpy as np

sys.path.insert(0, "/opt/trn_rl_repo")

B = 8
S = 1024
H = 1024
NH = 16
HD = 64
VALID = 512
P = 128
KC = H // P            # 8 contraction chunks of 128
QK_TILES = 2 * H // P  # 16 feature-major tiles for Q^T and K^T
TC = VALID // P        # 4 token chunks
VW = HD + 1            # V columns per head incl. ones column

_CACHE = {}


def _build(with_qkv_bias):
    import concourse.mybir as mybir
    import concourse.tile as tile
    from concourse import bacc
    from contextlib import ExitStack

    F32 = mybir.dt.float32
    F16 = mybir.dt.float16
    EXP = mybir.ActivationFunctionType.Exp

    nc = bacc.Bacc()
    xT = nc.declare_dram_parameter("xT", [P, KC, VALID], F16, isOutput=False)
    wqk = nc.declare_dram_parameter("wqk", [QK_TILES, P, KC, P], F16, isOutput=False)
    wv = nc.declare_dram_parameter("wv", [2, P, KC, 512], F16, isOutput=False)
    wo = nc.declare_dram_parameter("wo", [2, P, KC, 512], F16, isOutput=False)
    cos2 = nc.declare_dram_parameter("cos2", [P, VALID], F16, isOutput=False)
    sin2 = nc.declare_dram_parameter("sin2", [P, VALID], F16, isOutput=False)
    rot = nc.declare_dram_parameter("rot", [P, P], F16, isOutput=False)
    biask = nc.declare_dram_parameter("biask", [P, TC], F32, isOutput=False)
    ones16 = nc.declare_dram_parameter("ones16", [P, NH], F16, isOutput=False)
    if with_qkv_bias:
        qb_rope = nc.declare_dram_parameter("qb_rope", [P, QK_TILES, VALID], F32, isOutput=False)
        vbias = nc.declare_dram_parameter("vbias", [P, KC], F32, isOutput=False)
    out = nc.declare_dram_parameter("out", [VALID, H], F32, isOutput=True)

    with tile.TileContext(nc) as tc:
        with ExitStack() as ctx:
            consts = ctx.enter_context(tc.tile_pool(name="consts", bufs=1))
            xpool = ctx.enter_context(tc.tile_pool(name="x", bufs=1))
            # per-m weight tiles, loaded in consumption order, deep prefetch
            wqk_pool = ctx.enter_context(tc.tile_pool(name="wqk", bufs=6))
            wno_pool = ctx.enter_context(tc.tile_pool(name="wno", bufs=2))
            qsb_pool = ctx.enter_context(tc.tile_pool(name="qsb", bufs=3))
            qk_pool = ctx.enter_context(tc.tile_pool(name="qk", bufs=NH // 2 + NH))
            v_pool = ctx.enter_context(tc.tile_pool(name="v", bufs=TC))
            e_pool = ctx.enter_context(tc.tile_pool(name="e", bufs=NH * TC))
            ctx_pool = ctx.enter_context(tc.tile_pool(name="ctx", bufs=KC))
            tmp_pool = ctx.enter_context(tc.tile_pool(name="tmp", bufs=3))
            den_pool = ctx.enter_context(tc.tile_pool(name="den", bufs=4))
            rb_pool = ctx.enter_context(tc.tile_pool(name="rb", bufs=3))
            o_pool = ctx.enter_context(tc.tile_pool(name="o", bufs=3))
            proj_ps = ctx.enter_context(tc.tile_pool(name="pps", bufs=2, space="PSUM"))
            rot_ps = ctx.enter_context(tc.tile_pool(name="rps", bufs=2, space="PSUM"))
            s_ps = ctx.enter_context(tc.tile_pool(name="sps", bufs=2, space="PSUM"))
            c_ps = ctx.enter_context(tc.tile_pool(name="cps", bufs=2, space="PSUM"))

            # pin the activation table to natural_log_exp_and_others (set 6:
            # exp, ln, copy, identity all present) so the table-load pass
            # doesn't thrash between per-func sets
            nc.scalar.add_instruction(
                mybir.InstLoadActFuncSet(
                    name=nc.get_next_instruction_name(), ins=[], outs=[],
                    act_func_set_id=6,
                )
            )

            # Each HWDGE trigger costs ~0.65us on its sequencer, serialized
            # per ring.  Split x across both rings and issue the first
            # matmul's dependencies (x first half + first weight tile) as the
            # very first trigger on each ring so the PE starts ~10us earlier.
            xa1 = xpool.tile([P, 2, VALID], F16, tag="xa1", name="xa1")
            nc.scalar.dma_start(xa1[:], xT[:, 0:2, :])
            xa2 = xpool.tile([P, 2, VALID], F16, tag="xa2", name="xa2")
            nc.scalar.dma_start(xa2[:], xT[:, 2:4, :])

            # small consts on the scalar HWDGE ring; bulk loads on sync ring
            cos_t = consts.tile([P, VALID], F16, tag="cos")
            sin_t = consts.tile([P, VALID], F16, tag="sin")
            rot_t = consts.tile([P, P], F16, tag="rot")
            bias_t = consts.tile([P, TC], F32, tag="biask")
            ones_t = consts.tile([P, NH], F16, tag="ones16")
            nc.scalar.dma_start(cos_t[:], cos2[:])
            nc.scalar.dma_start(sin_t[:], sin2[:])
            nc.scalar.dma_start(rot_t[:], rot[:])
            nc.scalar.dma_start(bias_t[:], biask[:])
            nc.scalar.dma_start(ones_t[:], ones16[:])
            if with_qkv_bias:
                qb_t = consts.tile([P, QK_TILES, VALID], F32, tag="qb")
                nc.scalar.dma_start(qb_t[:], qb_rope[:])
                vb_t = consts.tile([P, KC], F32, tag="vb")
                nc.scalar.dma_start(vb_t[:], vbias[:])

            # first weight tile, then x second half, on the sync ring
            wm_tiles = {}
            wm0 = wqk_pool.tile([P, KC, P], F16, tag="wqk", name="wm0")
            nc.sync.dma_start(wm0[:], wqk[0])
            wm_tiles[0] = wm0
            xb = xpool.tile([P, 4, VALID], F16, tag="xb", name="xb")
            nc.sync.dma_start(xb[:], xT[:, 4:8, :])

            def x_sl(kc):
                if kc < 2:
                    return xa1[:, kc, :]
                if kc < 4:
                    return xa2[:, kc - 2, :]
                return xb[:, kc - 4, :]

            escale = 1.0 / np.sqrt(HD)
            qk_tiles = {}
            kpads = {}
            e_tiles = {}
            # zero-padded K-tile halves: full-128-partition lhsT for the
            # scores matmuls enables fast weight load (the zero half
            # multiplies the other head's q rows by 0)
            for i in range(NH // 2):
                ka = qk_pool.tile([P, VALID], F16, tag="qk", name=f"kpa{i}")
                kb = qk_pool.tile([P, VALID], F16, tag="qk", name=f"kpb{i}")
                nc.gpsimd.memset(ka[HD:, :], 0.0)
                nc.gpsimd.memset(kb[:HD, :], 0.0)
                kpads[i] = (ka, kb)

            def emit_rot(pend):
                q_sb, m = pend
                rp = rot_ps.tile([P, VALID], F32, tag="rps", name=f"rp{m}")
                nc.tensor.matmul(rp[:], rot_t[:], q_sb[:], start=True, stop=True)
                t1 = tmp_pool.tile([P, VALID], F16, tag="t1", name=f"t1_{m}")
                nc.vector.tensor_mul(t1[:], q_sb[:], cos_t[:])
                t2 = tmp_pool.tile([P, VALID], F16, tag="t2", name=f"t2_{m}")
                nc.vector.tensor_mul(t2[:], rp[:], sin_t[:])
                if m >= NH // 2:
                    # K tile: write the roped halves into the zero-padded pair
                    ka, kb = kpads[m - NH // 2]
                    if with_qkv_bias:
                        t3 = tmp_pool.tile([P, VALID], F32, tag="t3", name=f"t3_{m}")
                        nc.vector.tensor_add(t3[:], t1[:], t2[:])
                        nc.vector.tensor_add(t3[:], t3[:], qb_t[:, m, :])
                        nc.vector.tensor_copy(ka[:HD, :], t3[:HD, :])
                        nc.vector.tensor_copy(kb[HD:, :], t3[HD:, :])
                    else:
                        nc.vector.tensor_add(ka[:HD, :], t1[:HD, :], t2[:HD, :])
                        nc.vector.tensor_add(kb[HD:, :], t1[HD:, :], t2[HD:, :])
                else:
                    qkm = qk_pool.tile([P, VALID], F16, tag="qk", name=f"qk{m}")
                    if with_qkv_bias:
                        t3 = tmp_pool.tile([P, VALID], F32, tag="t3", name=f"t3_{m}")
                        nc.vector.tensor_add(t3[:], t1[:], t2[:])
                        nc.vector.tensor_add(qkm[:], t3[:], qb_t[:, m, :])
                    else:
                        nc.vector.tensor_add(qkm[:], t1[:], t2[:])
                    qk_tiles[m] = qkm

            def emit_scores(pair):
                qt = qk_tiles[pair]
                for h in (2 * pair, 2 * pair + 1):
                    kp = kpads[pair][h % 2]
                    for j in range(TC):
                        sp = s_ps.tile([P, VALID], F32, tag="sps", name=f"s{h}_{j}")
                        nc.tensor.matmul(
                            sp[:],
                            kp[:, j * P : (j + 1) * P],
                            qt[:],
                            start=True, stop=True,
                        )
                        ej = e_pool.tile([P, VALID], F16, tag="e", name=f"e{h}_{j}")
                        nc.scalar.activation(
                            ej[:], sp[:], EXP, bias=bias_t[:, j : j + 1], scale=escale
                        )
                        e_tiles[(h, j)] = ej

            # ---- Phase B: QK projection + RoPE + scores/exp, interleaved ----
            order = [m for pair in range(NH // 2) for m in (pair, NH // 2 + pair)]
            pend = None
            for m in order:
                if m in wm_tiles:
                    wm = wm_tiles[m]
                else:
                    wm = wqk_pool.tile([P, KC, P], F16, tag="wqk", name=f"wm{m}")
                    nc.sync.dma_start(wm[:], wqk[m])
                ps = proj_ps.tile([P, VALID], F32, tag="pps", name=f"ps{m}")
                for kc in range(KC):
                    nc.tensor.matmul(
                        ps[:], wm[:, kc, :], x_sl(kc),
                        start=(kc == 0), stop=(kc == KC - 1),
                    )
                q_sb = qsb_pool.tile([P, VALID], F16, tag="qsb", name=f"qsb{m}")
                nc.vector.tensor_copy(q_sb[:], ps[:])
                if pend is not None:
                    pm = pend[1]
                    emit_rot(pend)
                    if pm >= NH // 2:
                        emit_scores(pm - NH // 2)
                pend = (q_sb, m)
            emit_rot(pend)
            emit_scores(NH // 2 - 1)

            # ---- Phase C: V projection into token-major augmented layout ----
            v_tiles = [v_pool.tile([P, NH, VW], F16, tag="v", name=f"v{t}") for t in range(TC)]
            for t in range(TC):
                nc.scalar.copy(v_tiles[t][:, :, HD], ones_t[:])
            wvns = []
            for n in range(2):
                wvn = wno_pool.tile([P, KC, 512], F16, tag="wno", name=f"wv{n}")
                nc.sync.dma_start(wvn[:], wv[n])
                wvns.append(wvn)
            for t in range(TC):
                for n in range(2):
                    ps = proj_ps.tile([P, 512], F32, tag="pps", name=f"vps{n}{t}")
                    for kc in range(KC):
                        nc.tensor.matmul(
                            ps[:], x_sl(kc)[:, t * P : (t + 1) * P], wvns[n][:, kc, :],
                            start=(kc == 0), stop=(kc == KC - 1),
                        )
                    nc.scalar.copy(
                        v_tiles[t][:, 8 * n : 8 * n + 8, :HD],
                        ps[:].rearrange("p (h c) -> p h c", c=HD),
                    )

            # ---- Phase D: ctx matmuls + batched-denominator normalize ----
            ctx_tiles = [ctx_pool.tile([P, VALID], F16, tag="ctx", name=f"ctx{m}") for m in range(KC)]
            cps = {}
            rrs = {}
            NB = 2  # heads per psum-pool alternation group

            def emit_norm(h):
                rb = rb_pool.tile([HD, VALID], F32, tag="rb", name=f"rb{h}")
                rr = rrs.pop(h)
                nc.gpsimd.partition_broadcast(rb[:], rr[:])
                cp = cps.pop(h)
                dst = ctx_tiles[h // 2][(h % 2) * HD : (h % 2) * HD + HD, :]
                if with_qkv_bias:
                    tmpc = tmp_pool.tile([HD, VALID], F32, tag="tc", name=f"tc{h}")
                    nc.vector.tensor_mul(tmpc[:], cp[:HD, :], rb[:])
                    nc.scalar.activation(
                        dst, tmpc[:], mybir.ActivationFunctionType.Identity,
                        bias=vb_t[(h % 2) * HD : (h % 2) * HD + HD, h // 2 : h // 2 + 1],
                    )
                else:
                    nc.vector.tensor_mul(dst, cp[:HD, :], rb[:])

            for h in range(NH):
                # alternate between two PSUM pools (rot pool is idle now) so
                # 4 ctx accumulators stay live across the normalize pipeline
                cpool, ctag = [(c_ps, "cps"), (rot_ps, "rps")][(h // NB) % 2]
                cp = cpool.tile([VW, VALID], F32, tag=ctag, name=f"c{h}")
                for j in range(TC):
                    nc.tensor.matmul(
                        cp[:], v_tiles[j][:, h, :], e_tiles[(h, j)][:],
                        start=(j == 0), stop=(j == TC - 1),
                    )
                # reciprocal via scalar engine: 1/d = exp(-ln(d)); DVE's exact
                # reciprocal is ~6 cycles/element and would dominate the DVE
                lg = den_pool.tile([1, VALID], F32, tag="lg", name=f"lg{h}")
                nc.scalar.activation(lg[:], cp[HD : HD + 1, :],
                                     mybir.ActivationFunctionType.Ln)
                rr = den_pool.tile([1, VALID], F32, tag="rr", name=f"rr{h}")
                nc.scalar.activation(rr[:], lg[:],
                                     mybir.ActivationFunctionType.Exp, scale=-1.0)
                cps[h] = cp
                rrs[h] = rr
                if h % NB == NB - 1:
                    for hh in range(h - NB + 1, h + 1):
                        emit_norm(hh)

            # ---- Phase E: output projection ----
            for n in range(2):
                won = wno_pool.tile([P, KC, 512], F16, tag="wno", name=f"wo{n}")
                nc.sync.dma_start(won[:], wo[n])
                for t in range(TC):
                    pool_ = proj_ps if (t % 2 == 0) else s_ps
                    tag_ = "pps" if (t % 2 == 0) else "sps"
                    ps = pool_.tile([P, 512], F32, tag=tag_, name=f"ops{n}{t}")
                    for m in range(KC):
                        nc.tensor.matmul(
                            ps[:], ctx_tiles[m][:, t * P : (t + 1) * P], won[:, m, :],
                            start=(m == 0), stop=(m == KC - 1),
                        )
                    ot = o_pool.tile([P, 512], F32, tag="o", name=f"o{n}{t}")
                    nc.scalar.copy(ot[:], ps[:])
                    nc.sync.dma_start(
                        out[t * P : (t + 1) * P, n * 512 : (n + 1) * 512], ot[:]
                    )

    nc.compile()
    return nc


def _get_nc(with_qkv_bias):
    key = bool(with_qkv_bias)
    if key not in _CACHE:
        _CACHE[key] = _build(key)
    return _CACHE[key]


def _rot_matrix():
    # R such that (R.T @ q)[d] == rotate_half(q)[d], block-diagonal per head
    r64 = np.zeros((HD, HD), np.float32)
    half = HD // 2
    for d in range(half):
        r64[d + half, d] = -1.0  # dest d < 32 gets -q[d+32]
        r64[d, d + half] = 1.0   # dest d >= 32 gets  q[d-32]
    r = np.zeros((P, P), np.float32)
    r[:HD, :HD] = r64
    r[HD:, HD:] = r64
    return r


def _to_tiles_kxm(w, ncols):
    """(H, F) weight -> (F//ncols, P, KC, ncols) fp16, contiguous."""
    F = w.shape[1]
    t = w.reshape(KC, P, F // ncols, ncols).transpose(2, 1, 0, 3)
    return np.ascontiguousarray(t.astype(np.float16))


def kernel(hidden_states, cos, sin, attention_bias, qkv_w, qkv_b, o_w, o_b,
           indices, batch, seqlen, _trace=False):
    from concourse.bass_utils import run_bass_kernel_spmd

    hidden_states = np.asarray(hidden_states, dtype=np.float32)
    cos = np.asarray(cos, dtype=np.float32)
    sin = np.asarray(sin, dtype=np.float32)
    attention_bias = np.asarray(attention_bias, dtype=np.float32)
    qkv_w = np.asarray(qkv_w, dtype=np.float32)
    qkv_b = np.asarray(qkv_b, dtype=np.float32)
    o_w = np.asarray(o_w, dtype=np.float32)
    o_b = np.asarray(o_b, dtype=np.float32)
    indices = np.asarray(indices)
    batch = int(batch)
    seqlen = int(seqlen)

    with_bias = bool(np.any(qkv_b))

    pos = indices.astype(np.int64)
    b_of = pos // seqlen
    s_of = pos % seqlen

    wqk2 = _to_tiles_kxm(qkv_w[:, : 2 * H], P)        # (16, P, KC, P)
    wv2 = _to_tiles_kxm(qkv_w[:, 2 * H :], 512)       # (2, P, KC, 512)
    wo2 = _to_tiles_kxm(o_w, 512)                     # (2, P, KC, 512)
    rot = _rot_matrix().astype(np.float16)
    ones16 = np.ones((P, NH), np.float16)

    in_maps = []
    tok_idx = []
    for b in range(batch):
        idx = np.nonzero(b_of == b)[0]
        assert len(idx) == VALID, f"batch {b} has {len(idx)} valid tokens"
        tok_idx.append(idx)
        xT2 = np.ascontiguousarray(
            hidden_states[idx].T.reshape(KC, P, VALID).transpose(1, 0, 2)
            .astype(np.float16)
        )
        cosT = cos[idx, 0, :].T  # (HD, VALID)
        sinT = sin[idx, 0, :].T
        cos2 = np.ascontiguousarray(
            np.concatenate([cosT, cosT], axis=0).astype(np.float16))
        sin2 = np.ascontiguousarray(
            np.concatenate([sinT, sinT], axis=0).astype(np.float16))
        bias_b = attention_bias[b, 0, 0, s_of[idx]].astype(np.float32)  # (VALID,)
        biask = np.ascontiguousarray(bias_b.reshape(TC, P).T)  # (P, TC)
        m = {
            "xT": xT2, "wqk": wqk2, "wv": wv2, "wo": wo2,
            "cos2": cos2, "sin2": sin2, "rot": rot, "biask": biask,
            "ones16": ones16,
        }
        if with_bias:
            bq = qkv_b[: 2 * H]
            cos_full = np.tile(cosT, (2 * H // HD, 1))  # (2H, VALID)
            sin_full = np.tile(sinT, (2 * H // HD, 1))
            rot_bq = bq.reshape(-1, 2, HD // 2)[:, ::-1, :].copy()
            rot_bq[:, 0, :] *= -1.0
            rot_bq = rot_bq.reshape(-1)
            qb = (bq[:, None] * cos_full + rot_bq[:, None] * sin_full)
            qb = qb.reshape(QK_TILES, P, VALID).transpose(1, 0, 2)
            m["qb_rope"] = np.ascontiguousarray(qb.astype(np.float32))
            bv = qkv_b[2 * H :].astype(np.float32)
            m["vbias"] = np.ascontiguousarray(bv.reshape(KC, P).T)
        in_maps.append(m)

    nc = _get_nc(with_bias)
    res = run_bass_kernel_spmd(nc, in_maps, core_ids=list(range(B)), trace=_trace)

    T = hidden_states.shape[0]
    out_full = np.empty((T, H), np.float32)
    for b in range(batch):
        out_full[tok_idx[b]] = res.results[b]["out"]
    if np.any(o_b):
        out_full += o_b[None, :]
    if _trace:
        kernel.last_exec_time_ns = res.exec_time_ns
        kernel.last_results = res
    return out_full
